# revision 1
# baseline (speedup 1.0000x reference)
"""Trainium2 Bass kernel for the ClassicSDE problem.

Euler-Maruyama SDE integrator: batch 512 sharded data-parallel over 8
NeuronCores (64 samples/core). Per step, two 4-layer MLPs (drift
129->256->256->256->128, diffusion 129->256->256->256->2048) and a noise
einsum, 127 sequential steps.

Device layout: activations feature-major (features on partitions, batch on
free) so layers chain with zero transposes; weights are the stationary
matmul operand (bf16, FWL). The time-concat is folded into per-step bias
tables, LipSwish's 0.909 into downstream weights, dt into the drift output
weights. The noise einsum is algebraically fused into the PE: with
U[(n,j,r),b] = a3f[j*128+r,b]*bm[n,b] (one broadcast DVE multiply) and W4
reordered as W4nm[r,(n,j,h)] = W4f[j*128+r, h*16+n], the einsum becomes a
K=4096 matmul accumulated into the same PSUM bank as the drift update.
Brownian increments are precomputed on host (jax threefry is deterministic)
and DMA'd per step with a partition-broadcast access pattern.
"""

from contextlib import ExitStack

import numpy as np
import ml_dtypes

import concourse.bacc as bacc
import concourse.mybir as mybir
from concourse.tile import TileContext
from concourse.bass_utils import run_bass_kernel_spmd

NCORES = 8
DATA = 64
HID = 128
MLP = 256
NOISE = 16
LEN = 128
BPC = 512 // NCORES  # batch per core = 64
P = 128

BF16 = mybir.dt.bfloat16
F32 = mybir.dt.float32
SILU = mybir.ActivationFunctionType.Silu
MULT = mybir.AluOpType.mult
ADD = mybir.AluOpType.add


# ---------------------------------------------------------------- host prep

def _np(x):
    return np.asarray(x, dtype=np.float32)


def _chunk_cols(W):
    """Split (K, M) into (kc, mc) chunks of (<=128, 128) column blocks."""
    K, M = W.shape
    kcs = max(1, K // P)
    mcs = M // P
    out = []
    for kc in range(kcs):
        for mc in range(mcs):
            out.append(W[kc * P:(kc + 1) * P, mc * P:(mc + 1) * P])
    return out, kcs, mcs


def _prep(ts, initial, read_in_params, read_out_params, drift_params,
          diffusion_params, nsteps):
    """Returns (meta, shared host arrays, per-core in_maps)."""
    import jax

    ts = _np(ts)
    dts = ts[1:] - ts[:-1]
    dtbar = float(dts.mean())

    ri = [(_np(w), _np(b)) for w, b in read_in_params]
    dr = [(_np(w), _np(b)) for w, b in drift_params]
    df = [(_np(w), _np(b)) for w, b in diffusion_params]
    Wro, bro = _np(read_out_params[0]), _np(read_out_params[1])

    # All hidden-layer / output biases are structurally zero in this problem
    # (init_mlp uses zeros); only the folded t-row produces nonzero biases.
    for (w, b) in ri + dr[1:] + df[1:]:
        assert np.abs(b).max() == 0.0, "nonzero MLP bias unsupported"
    assert np.abs(bro).max() == 0.0

    S = 0.909

    # weight chunk packing (bf16): record column offsets
    cols = []
    offs = {}

    def pack(name, W):
        chunks, kcs, mcs = _chunk_cols(W)
        offs[name] = (len(cols) * P // 1, kcs, mcs)  # start col computed later
        offs[name] = (sum(c.shape[1] for c in cols), kcs, mcs)
        cols.extend(chunks)

    pack("ri1", ri[0][0])                    # (64, 256)
    pack("ri2", S * ri[1][0])
    pack("ri3", S * ri[2][0])
    pack("ri4", S * ri[3][0])                # (256, 128)
    pack("d1", dr[0][0][:HID])               # (128, 256), t-row folded to bias
    pack("d2", S * dr[1][0])
    pack("d3", S * dr[2][0])
    pack("d4", S * dtbar * dr[3][0])         # (256, 128)
    pack("f1", df[0][0][:HID])
    pack("f2", S * df[1][0])
    pack("f3", S * df[2][0])
    # diffusion L4 reordered for the fused einsum:
    # W4nm[r, (n*2+j)*128 + h] = S * W4f[j*128 + r, h*16 + n]
    W4f = df[3][0]                            # (256, 2048)
    w4 = S * W4f.reshape(2, P, HID, NOISE)    # (j, r, h, n)
    w4 = np.transpose(w4, (1, 3, 0, 2))       # (r, n, j, h)
    pack("w4nm", w4.reshape(P, 2 * NOISE * HID).astype(np.float32)
         .reshape(P, 32 * P))
    # fix: pack() chunks along K; w4 here is already (128, 4096) col-major
    # chunks, so chunking with K=128 gives 32 (128,128) blocks in order.

    wcols = sum(c.shape[1] for c in cols)
    wts = np.zeros((P, wcols), dtype=np.float32)
    o = 0
    for c in cols:
        wts[:c.shape[0], o:o + c.shape[1]] = c
        o += c.shape[1]
    wts = wts.astype(ml_dtypes.bfloat16)

    # fp32 bias/aux tensor: beff_d (2*nsteps), beff_f (2*nsteps), Wro (64)
    bcols = 2 * nsteps + 2 * nsteps + DATA
    bias = np.zeros((P, bcols), dtype=np.float32)
    boffs = {}
    # beff[r, m*nsteps + i] = b1[m*128+r] + t_i * W1[last_row, m*128+r]
    o = 0
    for name, (W1, b1) in (("d", dr[0]), ("f", df[0])):
        tab = b1[None, :] + ts[:nsteps, None] * W1[HID][None, :]  # (nsteps, 256)
        tab = tab.T.reshape(2, P, nsteps).transpose(1, 0, 2)      # (r, m, i)
        bias[:, o:o + 2 * nsteps] = tab.reshape(P, 2 * nsteps)
        boffs["beff_" + name] = o
        o += 2 * nsteps
    bias[:, o:o + DATA] = Wro
    boffs["wro"] = o

    # Brownian increments for all steps (threefry, matches the reference).
    cpu = jax.devices("cpu")[0]
    with jax.default_device(cpu):
        base_key = jax.random.key(42)
        bms = []
        for i in range(nsteps):
            k = jax.random.fold_in(base_key, i)
            bm = jax.random.normal(k, (NCORES * BPC, NOISE), np.float32)
            bms.append(np.asarray(bm) * np.sqrt(dts[i]))
    bms = np.stack(bms)                       # (nsteps, 512, 16)

    initial = _np(initial)                    # (512, 64)

    in_maps = []
    for c in range(NCORES):
        sl = slice(c * BPC, (c + 1) * BPC)
        init_tb = initial[sl].T.astype(ml_dtypes.bfloat16)         # (64, 64)
        # bmb[i, n*64 + b] = bm_i[b_global, n]
        bmb = bms[:, sl, :].transpose(0, 2, 1).reshape(nsteps, NOISE * BPC)
        in_maps.append({
            "wts": wts,
            "bias": bias,
            "init": np.ascontiguousarray(init_tb),
            "bmb": np.ascontiguousarray(bmb.astype(ml_dtypes.bfloat16)),
        })

    meta = dict(wcols=wcols, bcols=bcols, offs=offs, boffs=boffs,
                nsteps=nsteps)
    return meta, in_maps


# ------------------------------------------------------------- device build

def _build_nc(meta):
    nsteps = meta["nsteps"]
    offs = meta["offs"]
    boffs = meta["boffs"]
    nslots = nsteps + 1

    nc = bacc.Bacc(None, target_bir_lowering=False)
    wts_d = nc.dram_tensor("wts", [P, meta["wcols"]], BF16, kind="ExternalInput")
    bias_d = nc.dram_tensor("bias", [P, meta["bcols"]], F32, kind="ExternalInput")
    init_d = nc.dram_tensor("init", [DATA, BPC], BF16, kind="ExternalInput")
    bmb_d = nc.dram_tensor("bmb", [nsteps, NOISE * BPC], BF16, kind="ExternalInput")
    out_d = nc.dram_tensor("out", [DATA, nslots * BPC], F32, kind="ExternalOutput")

    with TileContext(nc) as tc, ExitStack() as ctx:
        const = ctx.enter_context(tc.tile_pool(name="const", bufs=1))
        actp = ctx.enter_context(tc.tile_pool(name="act", bufs=2))
        xp = ctx.enter_context(tc.tile_pool(name="xb", bufs=2))
        up = ctx.enter_context(tc.tile_pool(name="u", bufs=2))
        bmbp = ctx.enter_context(tc.tile_pool(name="bmb", bufs=4))
        outp = ctx.enter_context(tc.tile_pool(name="outp", bufs=1))
        psz = ctx.enter_context(tc.tile_pool(name="psz", bufs=4, space="PSUM"))
        pss = ctx.enter_context(tc.tile_pool(name="pss", bufs=2, space="PSUM"))
        psr = ctx.enter_context(tc.tile_pool(name="psr", bufs=2, space="PSUM"))

        wts_t = const.tile([P, meta["wcols"]], BF16)
        bias_t = const.tile([P, meta["bcols"]], F32)
        init_t = const.tile([DATA, BPC], BF16)
        xs_t = const.tile([P, nslots, BPC], F32)
        nc.sync.dma_start(out=wts_t[:], in_=wts_d[:])
        nc.sync.dma_start(out=bias_t[:], in_=bias_d[:])
        nc.sync.dma_start(out=init_t[:], in_=init_d[:])

        def wchunk(name, kc, mc, kpart=P):
            o, kcs, mcs = offs[name]
            return wts_t[0:kpart, o + (kc * mcs + mc) * P:
                         o + (kc * mcs + mc) * P + P]

        def mlp_layer(name, act_in, kpart=P, tag=None, bias_name=None,
                      step=None, out_bf16=True):
            """matmul layer + fused Silu eviction; returns activation tile."""
            _, kcs, mcs = offs[name]
            pz = psz.tile([P, mcs, BPC], F32, tag="z")
            for m in range(mcs):
                for k in range(kcs):
                    rhs = act_in if kcs == 1 else act_in[:, k, :]
                    nc.tensor.matmul(pz[:, m, :], wchunk(name, k, m, kpart),
                                     rhs, start=(k == 0), stop=(k == kcs - 1))
            a = actp.tile([P, mcs, BPC], BF16, tag=tag or name)
            if bias_name is None:
                nc.scalar.activation(a[:], pz[:], SILU, bias=0.0, scale=1.0)
            else:
                bo = boffs[bias_name]
                for m in range(mcs):
                    c = bo + m * nsteps + step
                    nc.scalar.activation(a[:, m, :], pz[:, m, :], SILU,
                                         bias=bias_t[:, c:c + 1], scale=1.0)
            return a

        # ---- prologue: x0 = read_in MLP(initial)
        a = mlp_layer("ri1", init_t[:], kpart=DATA)
        a = mlp_layer("ri2", a)
        a = mlp_layer("ri3", a)
        ps = pss.tile([P, BPC], F32, tag="s")
        for k in range(2):
            nc.tensor.matmul(ps[:], wchunk("ri4", k, 0), a[:, k, :],
                             start=(k == 0), stop=(k == 1))
        nc.scalar.copy(xs_t[:, 0, :], ps[:])
        xb = xp.tile([P, BPC], BF16, tag="xb")
        nc.vector.tensor_copy(xb[:], ps[:])

        # ---- time loop
        for i in range(nsteps):
            bmb_t = bmbp.tile([P, NOISE, BPC], BF16, tag="bmb")
            nc.sync.dma_start(
                out=bmb_t[:].rearrange("p n b -> p (n b)"),
                in_=bmb_d[i:i + 1, :].broadcast_to([P, NOISE * BPC]))

            # diffusion branch first (long pole: feeds U)
            a1f = mlp_layer("f1", xb[:], tag="a1f", bias_name="beff_f", step=i)
            a1d = mlp_layer("d1", xb[:], tag="a1d", bias_name="beff_d", step=i)
            a2f = mlp_layer("f2", a1f, tag="a2f")
            a2d = mlp_layer("d2", a1d, tag="a2d")
            a3f = mlp_layer("f3", a2f, tag="a3f")
            a3d = mlp_layer("d3", a2d, tag="a3d")

            # U[r, n, j, b] = a3f[r, j, b] * bmb[r(n), b]  (broadcast multiply)
            u_t = up.tile([P, NOISE, 2, BPC], BF16, tag="u")
            NQ = 4
            for q in range(NOISE // NQ):
                in0 = a3f[:].unsqueeze(1).broadcast_to([P, NQ, 2, BPC])
                in1 = bmb_t[:, q * NQ:(q + 1) * NQ, :].unsqueeze(2) \
                    .broadcast_to([P, NQ, 2, BPC])
                nc.vector.tensor_tensor(u_t[:, q * NQ:(q + 1) * NQ, :, :],
                                        in0, in1, MULT)

            # S = dt*drift + einsum, accumulated in one PSUM bank
            sp = pss.tile([P, BPC], F32, tag="s")
            nc.tensor.matmul(sp[:], wchunk("d4", 0, 0), a3d[:, 0, :],
                             start=True, stop=False)
            nc.tensor.matmul(sp[:], wchunk("d4", 1, 0), a3d[:, 1, :],
                             start=False, stop=False)
            o4, _, _ = offs["w4nm"]
            for cch in range(2 * NOISE):
                n, j = cch // 2, cch % 2
                nc.tensor.matmul(
                    sp[:], wts_t[:, o4 + cch * P:o4 + (cch + 1) * P],
                    u_t[:, n, j, :], start=False, stop=(cch == 2 * NOISE - 1))

            # x_{i+1} = x_i + S ; keep fp32 state, bf16 copy for matmuls
            nc.vector.tensor_tensor(xs_t[:, i + 1, :], sp[:], xs_t[:, i, :], ADD)
            xb = xp.tile([P, BPC], BF16, tag="xb")
            nc.vector.tensor_copy(xb[:], xs_t[:, i + 1, :])

        # ---- epilogue: readout  out[d, (t,b)] = Wro.T @ xs
        total = nslots * BPC
        out_t = outp.tile([DATA, total], F32)
        ro = boffs["wro"]
        o = 0
        k = 0
        while o < total:
            w = min(512, total - o)
            rp = psr.tile([DATA, 512], F32, tag="ro")
            nc.tensor.matmul(rp[:, 0:w], bias_t[:, ro:ro + DATA],
                             xs_t[:].rearrange("p s b -> p (s b)")[:, o:o + w],
                             start=True, stop=True)
            if k % 2 == 0:
                nc.scalar.copy(out_t[:, o:o + w], rp[:, 0:w])
            else:
                nc.vector.tensor_copy(out_t[:, o:o + w], rp[:, 0:w])
            o += w
            k += 1
        nc.sync.dma_start(out=out_d[:], in_=out_t[:])

    nc.compile()
    return nc


_CACHE = {}


def _get_nc(meta):
    key = (meta["nsteps"], meta["wcols"], meta["bcols"])
    if key not in _CACHE:
        _CACHE[key] = _build_nc(meta)
    return _CACHE[key]


# ------------------------------------------------------------------- entry

def kernel(ts, batch_size, nei, initial, read_in_params, read_out_params,
           drift_params, diffusion_params, nsteps=None, _return_runner=False):
    ts = _np(ts)
    lenth = int(np.asarray(nei).shape[1])
    if nsteps is None:
        nsteps = lenth - 1
    meta, in_maps = _prep(ts, initial, read_in_params, read_out_params,
                          drift_params, diffusion_params, nsteps)
    nc = _get_nc(meta)
    res = run_bass_kernel_spmd(nc, in_maps, core_ids=list(range(NCORES)))

    batch = NCORES * BPC
    nslots = nsteps + 1
    ys = np.empty((batch, nslots, DATA), dtype=np.float32)
    for c in range(NCORES):
        o = res.results[c]["out"].reshape(DATA, nslots, BPC)
        ys[c * BPC:(c + 1) * BPC] = o.transpose(2, 1, 0)
    out = np.empty((batch, nslots, 1 + DATA), dtype=np.float32)
    out[:, :, 0] = ts[None, :nslots]
    out[:, :, 1:] = ys
    return out


# revision 18
# speedup vs baseline: 5283.5102x; 5283.5102x over previous
"""Trainium2 Bass kernel for the ClassicSDE problem.

Euler-Maruyama SDE integrator: batch 512 sharded data-parallel over 8
NeuronCores (64 samples/core). Per step, two 4-layer MLPs (drift
129->256->256->256->128, diffusion 129->256->256->256->2048) and a noise
einsum, 127 sequential steps.

Device layout: activations feature-major (features on partitions, batch on
free) so layers chain with zero transposes; weights are the stationary
matmul operand (bf16, FWL). The time-concat is folded into per-step bias
tables, LipSwish's 0.909 into downstream weights, dt into the drift output
weights. The noise einsum is algebraically fused into the PE: with
U[(j,n,r),b] = a3f[j*128+r,b]*bm[n,b] (broadcast DVE multiplies) and W4
reordered as W4nm[r,(j,n,h)] = W4f[j*128+r, h*16+n], the einsum becomes a
K=4096 matmul accumulated into the same PSUM bank as the drift update.
Brownian increments are precomputed on host (jax threefry is deterministic)
and DMA'd per step with a partition-broadcast access pattern.
"""

from contextlib import ExitStack

import numpy as np
import ml_dtypes

import concourse.bacc as bacc
import concourse.mybir as mybir
from concourse.tile import TileContext
from concourse.bass_utils import run_bass_kernel_spmd

NCORES = 8
DATA = 64
HID = 128
MLP = 256
NOISE = 16
LEN = 128
BPC = 512 // NCORES  # batch per core = 64
P = 128

BF16 = mybir.dt.bfloat16
F32 = mybir.dt.float32
SILU = mybir.ActivationFunctionType.Silu
MULT = mybir.AluOpType.mult
ADD = mybir.AluOpType.add


# ---------------------------------------------------------------- host prep

def _np(x):
    return np.asarray(x, dtype=np.float32)


def _chunk_cols(W):
    """Split (K, M) into (kc, mc) chunks of (<=128, 128) column blocks."""
    K, M = W.shape
    kcs = max(1, K // P)
    mcs = M // P
    out = []
    for kc in range(kcs):
        for mc in range(mcs):
            out.append(W[kc * P:(kc + 1) * P, mc * P:(mc + 1) * P])
    return out, kcs, mcs


def _prep(ts, initial, read_in_params, read_out_params, drift_params,
          diffusion_params, nsteps):
    """Returns (meta, shared host arrays, per-core in_maps)."""
    import jax

    ts = _np(ts)
    dts = ts[1:] - ts[:-1]
    dtbar = float(dts.mean())

    ri = [(_np(w), _np(b)) for w, b in read_in_params]
    dr = [(_np(w), _np(b)) for w, b in drift_params]
    df = [(_np(w), _np(b)) for w, b in diffusion_params]
    Wro, bro = _np(read_out_params[0]), _np(read_out_params[1])

    # All hidden-layer / output biases are structurally zero in this problem
    # (init_mlp uses zeros); only the folded t-row produces nonzero biases.
    for (w, b) in ri + dr[1:] + df[1:]:
        assert np.abs(b).max() == 0.0, "nonzero MLP bias unsupported"
    assert np.abs(bro).max() == 0.0

    S = 0.909

    # weight chunk packing (bf16): record column offsets
    cols = []
    offs = {}

    def pack(name, W):
        chunks, kcs, mcs = _chunk_cols(W)
        offs[name] = (sum(c.shape[1] for c in cols), kcs, mcs)
        cols.extend(chunks)

    pack("ri1", ri[0][0])                    # (64, 256)
    pack("ri2", S * ri[1][0])
    pack("ri3", S * ri[2][0])
    pack("ri4", S * ri[3][0])                # (256, 128)
    pack("d1", dr[0][0][:HID])               # (128, 256), t-row folded to bias
    pack("d2", S * dr[1][0])
    pack("d3", S * dr[2][0])
    pack("d4", S * dtbar * dr[3][0])         # (256, 128)
    pack("f1", df[0][0][:HID])
    pack("f2", S * df[1][0])
    pack("f3", S * df[2][0])
    # diffusion L4 reordered for the fused einsum, j-major so the first
    # half only needs a3f chunk 0:
    # W4nm[r, (j*16+n)*128 + h] = S * W4f[j*128 + r, h*16 + n]
    W4f = df[3][0]                            # (256, 2048)
    w4 = S * W4f.reshape(2, P, HID, NOISE)    # (j, r, h, n)
    w4 = np.transpose(w4, (1, 0, 3, 2))       # (r, j, n, h)
    pack("w4nm", np.ascontiguousarray(w4).reshape(P, 2 * NOISE * HID))
    pack("ro", np.pad(Wro, ((0, 0), (0, P - DATA))))  # (128, 64) padded to 128

    wcols = sum(c.shape[1] for c in cols)
    wts = np.zeros((P, wcols), dtype=np.float32)
    o = 0
    for c in cols:
        wts[:c.shape[0], o:o + c.shape[1]] = c
        o += c.shape[1]
    wts = wts.astype(ml_dtypes.bfloat16)

    # fp32 bias tensor: beff[r, m*nsteps + i] = b1[m*128+r] + t_i*W1[-1, m*128+r]
    bcols = 4 * nsteps
    bias = np.zeros((P, bcols), dtype=np.float32)
    boffs = {}
    o = 0
    for name, (W1, b1) in (("f", df[0]), ("d", dr[0])):
        tab = b1[None, :] + ts[:nsteps, None] * W1[HID][None, :]  # (nsteps, 256)
        tab = tab.T.reshape(2, P, nsteps).transpose(1, 0, 2)      # (r, m, i)
        bias[:, o:o + 2 * nsteps] = tab.reshape(P, 2 * nsteps)
        boffs["beff_" + name] = o
        o += 2 * nsteps

    # Brownian increments for all steps (threefry, matches the reference).
    cpu = jax.devices("cpu")[0]
    with jax.default_device(cpu):
        base_key = jax.random.key(42)
        bms = []
        for i in range(nsteps):
            k = jax.random.fold_in(base_key, i)
            bm = jax.random.normal(k, (NCORES * BPC, NOISE), np.float32)
            bms.append(np.asarray(bm) * np.sqrt(dts[i]))
    bms = np.stack(bms)                       # (nsteps, 512, 16)

    initial = _np(initial)                    # (512, 64)

    in_maps = []
    for c in range(NCORES):
        sl = slice(c * BPC, (c + 1) * BPC)
        init_tb = initial[sl].T.astype(ml_dtypes.bfloat16)         # (64, 64)
        # bmb[i, n*64 + b] = bm_i[b_global, n]
        bmb = bms[:, sl, :].transpose(0, 2, 1).reshape(nsteps, NOISE * BPC)
        in_maps.append({
            "wts": wts,
            "bias": bias,
            "init": np.ascontiguousarray(init_tb),
            "bmb": np.ascontiguousarray(bmb.astype(ml_dtypes.bfloat16)),
        })

    meta = dict(wcols=wcols, bcols=bcols, offs=offs, boffs=boffs,
                nsteps=nsteps)
    return meta, in_maps


# ------------------------------------------------------------- device build

def _build_nc(meta):
    nsteps = meta["nsteps"]
    offs = meta["offs"]
    boffs = meta["boffs"]
    nslots = nsteps + 1

    nc = bacc.Bacc(None, target_bir_lowering=False)
    wts_d = nc.dram_tensor("wts", [P, meta["wcols"]], BF16, kind="ExternalInput")
    bias_d = nc.dram_tensor("bias", [P, meta["bcols"]], F32, kind="ExternalInput")
    init_d = nc.dram_tensor("init", [DATA, BPC], BF16, kind="ExternalInput")
    bmb_d = nc.dram_tensor("bmb", [nsteps, NOISE * BPC], BF16, kind="ExternalInput")
    out_d = nc.dram_tensor("out", [DATA, nslots * BPC], F32, kind="ExternalOutput")

    with TileContext(nc) as tc, ExitStack() as ctx:
        const = ctx.enter_context(tc.tile_pool(name="const", bufs=1))
        actp = ctx.enter_context(tc.tile_pool(name="act", bufs=2))
        up = ctx.enter_context(tc.tile_pool(name="u", bufs=2))
        bmbp = ctx.enter_context(tc.tile_pool(name="bmb", bufs=4))
        outp = ctx.enter_context(tc.tile_pool(name="outp", bufs=1))
        psz = ctx.enter_context(tc.tile_pool(name="psz", bufs=5, space="PSUM"))
        pss = ctx.enter_context(tc.tile_pool(name="pss", bufs=2, space="PSUM"))
        psr = ctx.enter_context(tc.tile_pool(name="psr", bufs=1, space="PSUM"))

        wts_t = const.tile([P, meta["wcols"]], BF16)
        bias_t = const.tile([P, meta["bcols"]], F32)
        init_t = const.tile([DATA, BPC], BF16)

        xs_t = const.tile([P, nslots, BPC], F32)
        xsb_t = const.tile([P, nslots, BPC], BF16)
        nc.sync.dma_start(out=wts_t[:], in_=wts_d[:])
        nc.sync.dma_start(out=bias_t[:], in_=bias_d[:])
        nc.sync.dma_start(out=init_t[:], in_=init_d[:])

        def wchunk(name, kc, mc, kpart=P):
            o, kcs, mcs = offs[name]
            return wts_t[0:kpart, o + (kc * mcs + mc) * P:
                         o + (kc * mcs + mc) * P + P]

        def mlp_layer(name, act_in, kpart=P, tag=None, bias_name=None,
                      step=None, out_bf16=True):
            """matmul layer + fused Silu eviction; returns activation tile."""
            _, kcs, mcs = offs[name]
            pz = psz.tile([P, mcs, BPC], F32, tag="z")
            for m in range(mcs):
                for k in range(kcs):
                    rhs = act_in if kcs == 1 else act_in[:, k, :]
                    nc.tensor.matmul(pz[:, m, :], wchunk(name, k, m, kpart),
                                     rhs, start=(k == 0), stop=(k == kcs - 1))
            a = actp.tile([P, mcs, BPC], BF16, tag=tag or name)
            if bias_name is None:
                nc.scalar.activation(a[:], pz[:], SILU, bias=0.0, scale=1.0)
            else:
                bo = boffs[bias_name]
                for m in range(mcs):
                    c = bo + m * nsteps + step
                    nc.scalar.activation(a[:, m, :], pz[:, m, :], SILU,
                                         bias=bias_t[:, c:c + 1], scale=1.0)
            return a

        # ---- prologue: x0 = read_in MLP(initial)
        a = mlp_layer("ri1", init_t[:], kpart=DATA)
        a = mlp_layer("ri2", a)
        a = mlp_layer("ri3", a)
        ps = pss.tile([P, BPC], F32, tag="s")
        for k in range(2):
            nc.tensor.matmul(ps[:], wchunk("ri4", k, 0), a[:, k, :],
                             start=(k == 0), stop=(k == 1))
        nc.scalar.copy(xs_t[:, 0, :], ps[:])
        nc.vector.tensor_copy(xsb_t[:, 0, :], ps[:])
        xb = xsb_t[:, 0, :]

        # readout out[d, (t,b)] = Wro.T @ xsb, emitted in 512-col groups;
        # groups whose slots are ready are issued inside the loop to fill
        # PE gaps, the rest in the epilogue.
        total = nslots * BPC
        out_t = outp.tile([DATA, total], F32)
        ro_o, _, _ = offs["ro"]
        xsb_flat = xsb_t[:].rearrange("p s b -> p (s b)")
        ro_state = {"o": 0, "k": 0}

        def readout_group(max_col):
            while ro_state["o"] < min(max_col, total):
                o = ro_state["o"]
                w = min(512, total - o)
                if o + w > max_col:
                    return
                rp = psr.tile([DATA, 512], F32, tag="ro")
                nc.tensor.matmul(rp[:, 0:w], wts_t[:, ro_o:ro_o + DATA],
                                 xsb_flat[:, o:o + w], start=True, stop=True)
                if ro_state["k"] % 2 == 0:
                    nc.scalar.copy(out_t[:, o:o + w], rp[:, 0:w])
                else:
                    nc.vector.tensor_copy(out_t[:, o:o + w], rp[:, 0:w])
                ro_state["o"] = o + w
                ro_state["k"] += 1

        # ---- time loop (repeat>1 is a perf-benchmark mode: re-runs the same
        # steps with identical instruction structure; output is then garbage)
        for i in [i for _ in range(meta.get("repeat", 1))
                  for i in range(nsteps)]:
            if i >= 7 and i % 8 == 7:
                readout_group((i - 6) * BPC)  # slots 0..i-6 are final
            bmb_t = bmbp.tile([P, NOISE, BPC], BF16, tag="bmb")
            nc.sync.dma_start(
                out=bmb_t[:].rearrange("p n b -> p (n b)"),
                in_=bmb_d[i:i + 1, :].broadcast_to([P, NOISE * BPC]))

            # Per-branch layers: diffusion first (feeds U), drift fills the
            # PE gaps during diffusion's activation hops. The per-step
            # t-bias enters via K=1 matmuls (beff row (x) ones).
            def layer1(bname, name, tag):
                z = psz.tile([P, 2, BPC], F32, tag="z")
                for m in range(2):
                    nc.tensor.matmul(z[:, m, :], wchunk(name, 0, m), xb,
                                     start=True, stop=True)
                a = actp.tile([P, 2, BPC], BF16, tag=tag)
                bo = boffs[bname]
                for m in range(2):
                    c = bo + m * nsteps + i
                    nc.scalar.activation(a[:, m, :], z[:, m, :], SILU,
                                         bias=bias_t[:, c:c + 1], scale=1.0)
                return a

            def layer(name, prev, tag):
                z = psz.tile([P, 2, BPC], F32, tag="z")
                for m in range(2):
                    for k in range(2):
                        nc.tensor.matmul(z[:, m, :], wchunk(name, k, m),
                                         prev[:, k, :],
                                         start=(k == 0), stop=(k == 1))
                a = actp.tile([P, 2, BPC], BF16, tag=tag)
                nc.scalar.activation(a[:], z[:], SILU, bias=0.0, scale=1.0)
                return a

            a1f = layer1("beff_f", "f1", "a1f")
            a1d = layer1("beff_d", "d1", "a1d")
            a2f = layer("f2", a1f, "a2f")
            a2d = layer("d2", a1d, "a2d")
            a3f = layer("f3", a2f, "a3f")
            a3d = layer("d3", a2d, "a3d")

            # U[r, j, n, b] = a3f[r, j, b] * bmb[r(n), b]  (broadcast multiply)
            # j=0 on DVE in eighths (early U-MM start), j=1 on GPSIMD halves
            u_t = up.tile([P, 2, NOISE, BPC], BF16, tag="u")
            NQ = NOISE // 4
            for nh in range(4):
                in0 = a3f[:, 0, :].unsqueeze(1).broadcast_to([P, NQ, BPC])
                in1 = bmb_t[:, nh * NQ:(nh + 1) * NQ, :]
                nc.vector.tensor_tensor(
                    u_t[:, 0, nh * NQ:(nh + 1) * NQ, :], in0, in1, MULT)
            NH = NOISE // 2
            for nh in range(2):
                in0 = a3f[:, 1, :].unsqueeze(1).broadcast_to([P, NH, BPC])
                in1 = bmb_t[:, nh * NH:(nh + 1) * NH, :]
                nc.vector.tensor_tensor(
                    u_t[:, 1, nh * NH:(nh + 1) * NH, :], in0, in1, MULT)

            # S-group: x_i via fp32 identity matmul first (in-stream), then
            # U j=0, drift L4 (a3d latency hides under U), U j=1.
            sp = pss.tile([P, BPC], F32, tag="s")
            o4, _, _ = offs["w4nm"]

            def u_mm(cch, start, stop):
                j, n = cch // NOISE, cch % NOISE
                nc.tensor.matmul(
                    sp[:], wts_t[:, o4 + cch * P:o4 + (cch + 1) * P],
                    u_t[:, j, n, :], start=start, stop=stop)

            for cch in range(NOISE):
                u_mm(cch, cch == 0, False)
            nc.tensor.matmul(sp[:], wchunk("d4", 0, 0), a3d[:, 0, :],
                             start=False, stop=False)
            nc.tensor.matmul(sp[:], wchunk("d4", 1, 0), a3d[:, 1, :],
                             start=False, stop=False)
            for cch in range(NOISE, 2 * NOISE):
                u_mm(cch, False, cch == 2 * NOISE - 1)

            # x_{i+1} = x_i + S: bf16 add on DVE (critical path -> next L1),
            # fp32 state add on ACT-free path (off critical path)
            nc.vector.tensor_tensor(xsb_t[:, i + 1, :], sp[:], xs_t[:, i, :], ADD)
            xb = xsb_t[:, i + 1, :]
            nc.vector.tensor_tensor(xs_t[:, i + 1, :], sp[:], xs_t[:, i, :], ADD)

        # ---- epilogue: finish remaining readout groups
        readout_group(total)
        nc.sync.dma_start(out=out_d[:], in_=out_t[:])

    nc.compile()
    return nc


_CACHE = {}


def _get_nc(meta):
    key = (meta["nsteps"], meta["wcols"], meta["bcols"])
    if key not in _CACHE:
        _CACHE[key] = _build_nc(meta)
    return _CACHE[key]


# ------------------------------------------------------------------- entry

def kernel(ts, batch_size, nei, initial, read_in_params, read_out_params,
           drift_params, diffusion_params, nsteps=None):
    ts = _np(ts)
    lenth = int(np.asarray(nei).shape[1])
    if nsteps is None:
        nsteps = lenth - 1
    meta, in_maps = _prep(ts, initial, read_in_params, read_out_params,
                          drift_params, diffusion_params, nsteps)
    nc = _get_nc(meta)
    res = run_bass_kernel_spmd(nc, in_maps, core_ids=list(range(NCORES)))

    batch = NCORES * BPC
    nslots = nsteps + 1
    ys = np.empty((batch, nslots, DATA), dtype=np.float32)
    for c in range(NCORES):
        o = res.results[c]["out"].reshape(DATA, nslots, BPC)
        ys[c * BPC:(c + 1) * BPC] = o.transpose(2, 1, 0)
    out = np.empty((batch, nslots, 1 + DATA), dtype=np.float32)
    out[:, :, 0] = ts[None, :nslots]
    out[:, :, 1:] = ys
    return out
